# revision 10
# baseline (speedup 1.0000x reference)
"""AttentionMemory kernel for Trainium2 (8 NeuronCores, batch-parallel).

Per batch element b (one per core):
    affinity = Mk[b]^T @ Qk[b] / sqrt(CK)        # [HW, HW], K = CK = 64
    out      = softmax(affinity, axis=0)          # over the memory axis

Per-core layout: affinity tiles [m=128 partitions, n=512 free] computed with
fp32r matmuls, two m-tiles packed per pass into disjoint PE row-groups
(tile_position (0,0)/(64,0)) with inputs replicated on both partition
halves; exp on ScalarE (PSUM->SBUF, scale=1/8 folded into the activation);
column sums via two ones-vector fp32r matmul chains over the partition
halves (row-packed, running concurrently); reciprocal on VectorE; the
reciprocal row is replicated across partitions with a GpSimd partition
broadcast; per-column normalize via VectorE tensor_tensor with a stride-0
broadcast read; one 8 MB DMA store per 512-column strip.
"""

import sys

for _p in ("/opt/trn_rl_repo", "/root/.axon_site/_ro/trn_rl_repo"):
    if _p not in sys.path:
        sys.path.insert(0, _p)

import numpy as np
import concourse.bass as bass
import concourse.tile as tile
from concourse import bacc, mybir
from concourse.bass_utils import run_bass_kernel_spmd

f32 = mybir.dt.float32
f32r = mybir.dt.float32r
EXP = mybir.ActivationFunctionType.Exp

B = 8
CK = 64
HW = 4096
P = 128
NT = HW // P  # 32 m-tiles
W = 512  # strip width = one PSUM bank of fp32
NS = HW // W  # 8 strips
SCALE = 1.0 / 8.0  # 1/sqrt(CK)

_cache = {}


def _build():
    nc = bacc.Bacc("TRN2", target_bir_lowering=False, debug=False, num_devices=B)
    mk = nc.dram_tensor("mk", [CK, HW], f32, kind="ExternalInput")
    qk = nc.dram_tensor("qk", [CK, HW], f32, kind="ExternalInput")
    out = nc.dram_tensor("out", [HW, HW], f32, kind="ExternalOutput")

    with tile.TileContext(nc) as tc:
        with (
            tc.tile_pool(name="const", bufs=1) as constp,
            tc.tile_pool(name="stage", bufs=1) as stage,
            tc.tile_pool(name="inp", bufs=1) as inp,
            tc.tile_pool(name="strips", bufs=4) as strips,
            tc.tile_pool(name="small", bufs=2) as small,
            tc.tile_pool(name="mmps", bufs=2, space="PSUM") as mmps,
            tc.tile_pool(name="sumps", bufs=1, space="PSUM") as sumps,
            tc.tile_pool(name="rbps", bufs=2, space="PSUM") as rbps,
        ):
            ones32 = constp.tile([P, 1], f32)
            nc.vector.memset(ones32[:], 1.0)
            ones_k = constp.tile([P, 1], f32r)
            with nc.allow_low_precision(reason="ones are exact in f32r"):
                nc.vector.tensor_copy(ones_k[:], ones32[:])
            ones_m = constp.tile([1, P], f32)
            nc.vector.memset(ones_m[:], 1.0)

            # inputs replicated onto both partition halves, rounded to f32r
            mk_r = inp.tile([P, HW], f32r, tag="mkr")
            qk_r = inp.tile([P, HW], f32r, tag="qkr")
            for dst, src in ((mk_r, mk), (qk_r, qk)):
                st = stage.tile([P, HW], f32, tag="stage")
                nc.sync.dma_start(st[0:CK, :], src[:])
                nc.sync.dma_start(st[CK:P, :], src[:])
                with nc.allow_low_precision(reason="f32r matmul inputs"):
                    nc.vector.tensor_copy(dst[:], st[:])

            # out[t*128 + p, n] viewed as [p, t, n]
            out4 = out[:].rearrange("(t p) n -> p t n", p=P)

            HT = NT // 2  # tiles per half-strip
            for s in range(NS):
                er_h0 = strips.tile([P, HT, W], f32r, tag="er")
                er_h1 = strips.tile([P, HT, W], f32r, tag="er")
                halves = [er_h0, er_h1]
                chain_a = sumps.tile([1, W], f32, tag="ca")
                chain_b = sumps.tile([1, W], f32, tag="cb")
                ns = slice(s * W, (s + 1) * W)

                def emit_sums(t):
                    er = halves[t // HT]
                    tt = t % HT
                    nc.tensor.matmul(
                        chain_a[:],
                        ones_k[0:CK, :],
                        er[0:CK, tt, :],
                        start=(t == 0),
                        stop=(t == NT - 1),
                        tile_position=(0, 0),
                    )
                    nc.tensor.matmul(
                        chain_b[:],
                        ones_k[CK:P, :],
                        er[CK:P, tt, :],
                        start=(t == 0),
                        stop=(t == NT - 1),
                        tile_position=(CK, 0),
                    )

                for g in range(NT // 2):
                    ta, tb = 2 * g, 2 * g + 1
                    er = halves[ta // HT]
                    tt = ta % HT
                    pair = mmps.tile([P, 2 * W], f32, tag="pair")
                    nc.tensor.matmul(
                        pair[:, 0:W],
                        mk_r[0:CK, ta * P : (ta + 1) * P],
                        qk_r[0:CK, ns],
                        start=True,
                        stop=True,
                        tile_position=(0, 0),
                    )
                    nc.tensor.matmul(
                        pair[:, W : 2 * W],
                        mk_r[CK:P, tb * P : (tb + 1) * P],
                        qk_r[CK:P, ns],
                        start=True,
                        stop=True,
                        tile_position=(CK, 0),
                    )
                    nc.scalar.activation(
                        er[:, tt : tt + 2, :].rearrange("p t j -> p (t j)"),
                        pair[:],
                        EXP,
                        scale=SCALE,
                    )
                    # sums for the previous pair (ACT for it already done)
                    if g > 0:
                        emit_sums(2 * g - 2)
                        emit_sums(2 * g - 1)
                emit_sums(NT - 2)
                emit_sums(NT - 1)

                # combine the two partial-sum chains, reciprocal, broadcast
                sa = small.tile([1, W], f32, tag="sa")
                nc.scalar.copy(sa[:], chain_a[:])
                ssum = small.tile([1, W], f32, tag="ssum")
                nc.vector.tensor_add(ssum[:], sa[:], chain_b[:])
                r_sb = small.tile([1, W], f32, tag="r")
                scratch = small.tile([1, W], f32, tag="sa")
                nc.vector.reciprocal_approx_accurate(r_sb[:], ssum[:], scratch[:])
                # replicate r across partitions with a K=1 fp32 ones matmul
                rb = rbps.tile([P, W], f32, tag="rb")
                nc.tensor.matmul(rb[:], ones_m[:], r_sb[:], start=True, stop=True)

                # normalize in place (stride-0 broadcast read of rb), per half,
                # then store each half as soon as it is ready
                rb1 = rb[:].rearrange("p (t j) -> p t j", t=1)
                for h, er in enumerate(halves):
                    with nc.allow_low_precision(reason="output via f32r tile"):
                        for c in range(HT // 8):
                            nc.vector.tensor_mul(
                                er[:, 8 * c : 8 * c + 8, :],
                                er[:, 8 * c : 8 * c + 8, :],
                                rb1.broadcast_to([P, 8, W]),
                            )
                    nc.sync.dma_start(
                        out=out4[:, h * HT : (h + 1) * HT, ns],
                        in_=er[:].bitcast(f32),
                    )

    nc.compile()
    return nc


def kernel(Mk, Qk):
    Mk = np.asarray(Mk)
    Qk = np.asarray(Qk)
    assert Mk.shape == (B, CK, 64, 64) and Qk.shape == (B, CK, 64, 64)
    nc = _cache.get("nc")
    if nc is None:
        nc = _cache["nc"] = _build()
    in_maps = [
        {
            "mk": np.ascontiguousarray(Mk[b].reshape(CK, HW), dtype=np.float32),
            "qk": np.ascontiguousarray(Qk[b].reshape(CK, HW), dtype=np.float32),
        }
        for b in range(B)
    ]
    res = run_bass_kernel_spmd(nc, in_maps, list(range(B)))
    return np.stack([res.results[b]["out"] for b in range(B)], axis=0)


# revision 12
# speedup vs baseline: 1.1119x; 1.1119x over previous
"""AttentionMemory kernel for Trainium2 (8 NeuronCores, batch-parallel).

Per batch element b (one per core):
    affinity = Mk[b]^T @ Qk[b] / sqrt(CK)        # [HW, HW], K = CK = 64
    out      = softmax(affinity, axis=0)          # over the memory axis

Per-core layout: affinity tiles [m=128 partitions, n=512 free] computed with
fp32r matmuls, two m-tiles packed per pass into disjoint PE row-groups
(tile_position (0,0)/(64,0)) with inputs replicated on both partition
halves; exp on ScalarE (PSUM->SBUF, scale=1/8 folded into the activation);
column sums via two ones-vector fp32r matmul chains over the partition
halves (row-packed, running concurrently); reciprocal on VectorE; the
reciprocal row is replicated across partitions with a GpSimd partition
broadcast; per-column normalize via VectorE tensor_tensor with a stride-0
broadcast read; one 8 MB DMA store per 512-column strip.
"""

import sys

for _p in ("/opt/trn_rl_repo", "/root/.axon_site/_ro/trn_rl_repo"):
    if _p not in sys.path:
        sys.path.insert(0, _p)

import numpy as np
import concourse.bass as bass
import concourse.tile as tile
from concourse import bacc, mybir
from concourse.bass_utils import run_bass_kernel_spmd

f32 = mybir.dt.float32
f32r = mybir.dt.float32r
EXP = mybir.ActivationFunctionType.Exp

B = 8
CK = 64
HW = 4096
P = 128
NT = HW // P  # 32 m-tiles
W = 512  # strip width = one PSUM bank of fp32
NS = HW // W  # 8 strips
SCALE = 1.0 / 8.0  # 1/sqrt(CK)

_cache = {}


def _build():
    nc = bacc.Bacc("TRN2", target_bir_lowering=False, debug=False, num_devices=B)
    mk = nc.dram_tensor("mk", [CK, HW], f32, kind="ExternalInput")
    qk = nc.dram_tensor("qk", [CK, HW], f32, kind="ExternalInput")
    out = nc.dram_tensor("out", [HW, HW], f32, kind="ExternalOutput")

    with tile.TileContext(nc) as tc:
        with (
            tc.tile_pool(name="const", bufs=1) as constp,
            tc.tile_pool(name="stage", bufs=1) as stage,
            tc.tile_pool(name="inp", bufs=1) as inp,
            tc.tile_pool(name="strips", bufs=4) as strips,
            tc.tile_pool(name="small", bufs=2) as small,
            tc.tile_pool(name="mmps", bufs=2, space="PSUM") as mmps,
            tc.tile_pool(name="sumps", bufs=2, space="PSUM") as sumps,
        ):
            ones32 = constp.tile([P, 1], f32)
            nc.vector.memset(ones32[:], 1.0)
            ones_k = constp.tile([P, 1], f32r)
            with nc.allow_low_precision(reason="ones are exact in f32r"):
                nc.vector.tensor_copy(ones_k[:], ones32[:])
            ones_m = constp.tile([1, P], f32)
            nc.vector.memset(ones_m[:], 1.0)

            # inputs replicated onto both partition halves, rounded to f32r
            mk_r = inp.tile([P, HW], f32r, tag="mkr")
            qk_r = inp.tile([P, HW], f32r, tag="qkr")
            for dst, src in ((mk_r, mk), (qk_r, qk)):
                st = stage.tile([P, HW], f32, tag="stage")
                nc.sync.dma_start(st[0:CK, :], src[:])
                nc.sync.dma_start(st[CK:P, :], src[:])
                with nc.allow_low_precision(reason="f32r matmul inputs"):
                    nc.vector.tensor_copy(dst[:], st[:])

            # out[t*128 + p, n] viewed as [p, t, n]
            out4 = out[:].rearrange("(t p) n -> p t n", p=P)

            HT = NT // 2  # tiles per half-strip
            NPAIR = NT // 2  # m-tile pairs per strip
            LAG = 2  # sum-chain lag, in pairs

            strip_state = {}

            def ensure_strip(s):
                if s not in strip_state:
                    er_h0 = strips.tile([P, HT, W], f32r, tag="er")
                    er_h1 = strips.tile([P, HT, W], f32r, tag="er")
                    ca = sumps.tile([1, W], f32, tag="ca")
                    cb = sumps.tile([1, W], f32, tag="cb")
                    strip_state[s] = ([er_h0, er_h1], ca, cb)
                return strip_state[s]

            def emit_main_pair(k):
                s, g = divmod(k, NPAIR)
                halves, _, _ = ensure_strip(s)
                ta, tb = 2 * g, 2 * g + 1
                er = halves[ta // HT]
                tt = ta % HT
                ns = slice(s * W, (s + 1) * W)
                pair = mmps.tile([P, 2 * W], f32, tag="pair")
                nc.tensor.matmul(
                    pair[:, 0:W],
                    mk_r[0:CK, ta * P : (ta + 1) * P],
                    qk_r[0:CK, ns],
                    start=True,
                    stop=True,
                    tile_position=(0, 0),
                )
                nc.tensor.matmul(
                    pair[:, W : 2 * W],
                    mk_r[CK:P, tb * P : (tb + 1) * P],
                    qk_r[CK:P, ns],
                    start=True,
                    stop=True,
                    tile_position=(CK, 0),
                )
                nc.scalar.activation(
                    er[:, tt : tt + 2, :].rearrange("p t j -> p (t j)"),
                    pair[:],
                    EXP,
                    scale=SCALE,
                )

            def emit_sum(s, t):
                halves, ca, cb = strip_state[s]
                er = halves[t // HT]
                tt = t % HT
                nc.tensor.matmul(
                    ca[:],
                    ones_k[0:CK, :],
                    er[0:CK, tt, :],
                    start=(t == 0),
                    stop=(t == NT - 1),
                    tile_position=(0, 0),
                )
                nc.tensor.matmul(
                    cb[:],
                    ones_k[CK:P, :],
                    er[CK:P, tt, :],
                    start=(t == 0),
                    stop=(t == NT - 1),
                    tile_position=(CK, 0),
                )

            def emit_tail(s):
                halves, ca, cb = strip_state.pop(s)
                ns = slice(s * W, (s + 1) * W)
                # combine the two partial-sum chains, reciprocal, broadcast
                sa = small.tile([1, W], f32, tag="sa")
                nc.scalar.copy(sa[:], ca[:])
                ssum = small.tile([1, W], f32, tag="ssum")
                nc.vector.tensor_add(ssum[:], sa[:], cb[:])
                r_sb = small.tile([1, W], f32, tag="r")
                scratch = small.tile([1, W], f32, tag="sa")
                nc.vector.reciprocal_approx_accurate(r_sb[:], ssum[:], scratch[:])
                rb = small.tile([P, W], f32, tag="rb")
                nc.gpsimd.partition_broadcast(rb[:], r_sb[:])
                # normalize in place (stride-0 broadcast read of rb), per half,
                # then store each half as soon as it is ready
                rb1 = rb[:].rearrange("p (t j) -> p t j", t=1)
                for h, er in enumerate(halves):
                    with nc.allow_low_precision(reason="output via f32r tile"):
                        for c in range(HT // 8):
                            nc.vector.tensor_mul(
                                er[:, 8 * c : 8 * c + 8, :],
                                er[:, 8 * c : 8 * c + 8, :],
                                rb1.broadcast_to([P, 8, W]),
                            )
                    nc.sync.dma_start(
                        out=out4[:, h * HT : (h + 1) * HT, ns],
                        in_=er[:].bitcast(f32),
                    )

            for k in range(NS * NPAIR + LAG):
                if k < NS * NPAIR:
                    emit_main_pair(k)
                if k >= LAG:
                    s, g = divmod(k - LAG, NPAIR)
                    emit_sum(s, 2 * g)
                    emit_sum(s, 2 * g + 1)
                    if g == NPAIR - 1:
                        emit_tail(s)

    nc.compile()
    return nc


def kernel(Mk, Qk):
    Mk = np.asarray(Mk)
    Qk = np.asarray(Qk)
    assert Mk.shape == (B, CK, 64, 64) and Qk.shape == (B, CK, 64, 64)
    nc = _cache.get("nc")
    if nc is None:
        nc = _cache["nc"] = _build()
    in_maps = [
        {
            "mk": np.ascontiguousarray(Mk[b].reshape(CK, HW), dtype=np.float32),
            "qk": np.ascontiguousarray(Qk[b].reshape(CK, HW), dtype=np.float32),
        }
        for b in range(B)
    ]
    res = run_bass_kernel_spmd(nc, in_maps, list(range(B)))
    return np.stack([res.results[b]["out"] for b in range(B)], axis=0)
